# revision 4
# baseline (speedup 1.0000x reference)
"""Trainium2 Bass kernel for nn_AlphaEstimate (8-core data-parallel).

Math (per row r of tb=8192 rows, sample s<16, agent a<8):
    c[r,s,a]   = sum_{a'} A[r,s,a,a'] * q[r,a']          (A: fixed perm constant)
    pre[r,s,a,e] = c[r,s,a]*w1_0[r,e] + q[r,a]*w1_1[r,e] + b1[r,e]
    h          = elu(pre);  elu(x)+1 = min(exp(x),1) + relu(x)
    y[r,s,a]   = sum_e (h+1)*w2[r,e] + b2[r] - sum_e w2[r,e]
    alpha[r,a] = mean_s |y|
where w1 = |hyper1(states)|, b1 = states@b1_k+b1_b, w2 = |hyper2(states)|,
b2 = hyper3(states). Data-parallel: 1024 rows per core, no collectives.
"""

import sys

for _p in ("/opt/trn_rl_repo", "/opt/pypackages"):
    if _p not in sys.path:
        sys.path.append(_p)

import numpy as np

N, T, B, S = 8, 128, 64, 16
D, E, HH = 128, 64, 128
TB = T * B            # 8192
NCORES = 8
RPC = TB // NCORES    # 1024 rows per core
P = 128               # partition tile
NTILES = RPC // P     # 8 tiles per core
SA = S * N            # 128

_STATE = {}


def _perm_A():
    """Constant coalition matrix A[r, s*N*N + a*N + a'] (input independent)."""
    import jax

    cpu = jax.devices("cpu")[0]
    with jax.default_device(cpu):
        keys = jax.random.split(jax.random.key(42), TB * S)
        pos = jax.vmap(lambda k: jax.random.permutation(k, N))(keys)
        pos = np.asarray(jax.device_get(pos)).reshape(TB, S, N)
    mask = pos[:, :, None, :] < pos[:, :, :, None]          # [tb,s,a,a']
    A = mask.astype(np.float32) / np.maximum(pos, 1).astype(np.float32)[:, :, :, None]
    return np.ascontiguousarray(A.reshape(TB, S * N * N))


def _build():
    import concourse.bacc as bacc
    import concourse.tile as tile
    import concourse.mybir as mybir
    from concourse import masks

    dt = mybir.dt.float32
    AF = mybir.ActivationFunctionType
    OP = mybir.AluOpType
    AX = mybir.AxisListType

    nc = bacc.Bacc("TRN2", target_bir_lowering=False, debug=False,
                   num_devices=NCORES)

    def din(name, shape):
        return nc.dram_tensor(name, list(shape), dt, kind="ExternalInput").ap()

    statesT = din("statesT", (D, RPC))
    qin = din("q", (RPC, N))
    amat = din("amat", (RPC, S * N * N))
    hw1_k1 = din("hw1_k1", (D, HH))
    hw1_b1 = din("hw1_b1", (1, HH))
    hw1_k2 = din("hw1_k2", (HH, 2 * E))
    hw1_b2 = din("hw1_b2", (1, 2 * E))
    b1_k = din("b1_k", (D, E))
    b1_b = din("b1_b", (1, E))
    hw2_k1 = din("hw2_k1", (D, HH))
    hw2_b1 = din("hw2_b1", (1, HH))
    hw2_k2 = din("hw2_k2", (HH, E))
    hw2_b2 = din("hw2_b2", (1, E))
    hb2_k1 = din("hb2_k1", (D, E))
    hb2_b1 = din("hb2_b1", (1, E))
    hb2_k2 = din("hb2_k2", (E, 1))
    hb2_b2 = din("hb2_b2", (1, 1))
    out = nc.dram_tensor("alpha_out", [RPC, N], dt, kind="ExternalOutput").ap()

    with tile.TileContext(nc) as tc:
        with (
            tc.tile_pool(name="const", bufs=1) as cp,
            tc.tile_pool(name="big", bufs=2) as bp,
            tc.tile_pool(name="small", bufs=3) as sp,
            tc.tile_pool(name="psum", bufs=6, space="PSUM") as pp,
        ):
            ident = cp.tile([P, P], dt)
            masks.make_identity(nc, ident[:])
            ones = cp.tile([1, P], dt)
            nc.gpsimd.memset(ones[:], 1.0)

            stT = cp.tile([D, RPC], dt)
            nc.sync.dma_start(stT[:], statesT)

            def wtile(ap, shape, _n=[0]):
                _n[0] += 1
                t = cp.tile(list(shape), dt, tag=f"wt{_n[0]}")
                nc.sync.dma_start(t[:], ap)
                return t

            W11 = wtile(hw1_k1, (D, HH)); Bi11 = wtile(hw1_b1, (1, HH))
            W12 = wtile(hw1_k2, (HH, 2 * E)); Bi12 = wtile(hw1_b2, (1, 2 * E))
            Wb1 = wtile(b1_k, (D, E)); Bib1 = wtile(b1_b, (1, E))
            W21 = wtile(hw2_k1, (D, HH)); Bi21 = wtile(hw2_b1, (1, HH))
            W22 = wtile(hw2_k2, (HH, E)); Bi22 = wtile(hw2_b2, (1, E))
            W31 = wtile(hb2_k1, (D, E)); Bi31 = wtile(hb2_b1, (1, E))
            W32 = wtile(hb2_k2, (E, 1)); Bi32 = wtile(hb2_b2, (1, 1))

            for t in range(NTILES):
                r0 = t * P
                lhs_states = stT[:, r0:r0 + P]      # (d=128, rows=128)

                def layer(wt, bi, width, act, tag, accum=None, lhsT=None):
                    ps = pp.tile([P, width], dt, tag="ps")
                    nc.tensor.matmul(ps[:], lhsT if lhsT is not None else lhs_states,
                                     wt[:], start=True, stop=False)
                    nc.tensor.matmul(ps[:], ones[:], bi[:], start=False, stop=True)
                    sb = sp.tile([P, width], dt, tag=tag)
                    if accum is not None:
                        nc.scalar.activation(sb[:], ps[:], act, accum_out=accum)
                    else:
                        nc.scalar.activation(sb[:], ps[:], act)
                    return sb

                def transp(sb, width):
                    ps = pp.tile([width, P], dt, tag="ps")
                    nc.tensor.transpose(ps[:], sb[:], ident[:])
                    sbT = sp.tile([width, P], dt, tag=f"tr{width}")
                    nc.scalar.activation(sbT[:], ps[:], AF.Copy)
                    return sbT

                # hypernets (feature outputs in (row, feat) orientation)
                h1 = layer(W11, Bi11, HH, AF.Relu, "hh")
                h1T = transp(h1, HH)
                w1 = layer(W12, Bi12, 2 * E, AF.Abs, "w1", lhsT=h1T[:])
                b1 = layer(Wb1, Bib1, E, AF.Copy, "b1")
                h2 = layer(W21, Bi21, HH, AF.Relu, "hh")
                h2T = transp(h2, HH)
                w2sum = sp.tile([P, 1], dt, tag="w2sum")
                w2 = layer(W22, Bi22, E, AF.Abs, "w2", accum=w2sum[:], lhsT=h2T[:])
                h3 = layer(W31, Bi31, E, AF.Relu, "h3")
                h3T = transp(h3, E)
                b2 = layer(W32, Bi32, 1, AF.Copy, "b2", lhsT=h3T[:])

                w1_0 = w1[:, 0:E]
                w1_1 = w1[:, E:2 * E]

                # coalition means: c = reduce_a'(A * q)
                At = bp.tile([P, S * N * N], dt, tag="amat")
                nc.sync.dma_start(At[:], amat[r0:r0 + P, :])
                qt = sp.tile([P, N], dt, tag="qt")
                nc.sync.dma_start(qt[:], qin[r0:r0 + P, :])
                prod = bp.tile([P, S * N * N], dt, tag="prod")
                nc.vector.tensor_tensor(
                    prod[:].rearrange("p (sa ap) -> p sa ap", ap=N),
                    At[:].rearrange("p (sa ap) -> p sa ap", ap=N),
                    qt[:].unsqueeze(1).broadcast_to((P, SA, N)),
                    OP.mult)
                c = sp.tile([P, SA], dt, tag="c")
                nc.vector.tensor_reduce(
                    c[:], prod[:].rearrange("p (sa ap) -> p sa ap", ap=N),
                    AX.X, OP.add)

                # qw[a,e] = q_a*w1_1 + b1
                qw = sp.tile([P, N * E], dt, tag="qw")
                qwv = qw[:].rearrange("p (a e) -> p a e", e=E)
                nc.vector.tensor_tensor(
                    qwv,
                    qt[:].unsqueeze(2).broadcast_to((P, N, E)),
                    w1_1.unsqueeze(1).broadcast_to((P, N, E)),
                    OP.mult)
                nc.vector.tensor_tensor(
                    qwv, qwv,
                    b1[:].unsqueeze(1).broadcast_to((P, N, E)),
                    OP.add)

                # pre = c (x) w1_0 + qw  (sa, e) layout
                pre = bp.tile([P, SA * E], dt, tag="pre")
                pv = pre[:].rearrange("p (sa e) -> p sa e", e=E)
                nc.vector.tensor_tensor(
                    pv,
                    c[:].unsqueeze(2).broadcast_to((P, SA, E)),
                    w1_0.unsqueeze(1).broadcast_to((P, SA, E)),
                    OP.mult)
                pv4 = pre[:].rearrange("p (s a e) -> p s a e", a=N, e=E)
                nc.vector.tensor_tensor(
                    pv4, pv4,
                    qw[:].rearrange("p (a e) -> p a e", e=E)
                         .unsqueeze(1).broadcast_to((P, S, N, E)),
                    OP.add)

                # h+1 = min(exp(pre),1) + relu(pre)
                eb = bp.tile([P, SA * E], dt, tag="exp")
                nc.scalar.activation(eb[:], pre[:], AF.Exp)
                nc.scalar.activation(pre[:], pre[:], AF.Relu)
                nc.vector.scalar_tensor_tensor(
                    eb[:], eb[:], 1.0, pre[:], OP.min, OP.add)

                # y = reduce_e((h+1) * w2); yf = y + b2 - w2sum
                ev = eb[:].rearrange("p (sa e) -> p sa e", e=E)
                nc.vector.tensor_tensor(
                    ev, ev, w2[:].unsqueeze(1).broadcast_to((P, SA, E)), OP.mult)
                y = sp.tile([P, SA], dt, tag="y")
                nc.vector.tensor_reduce(y[:], ev, AX.X, OP.add)
                nc.vector.tensor_scalar(y[:], y[:], b2[:], w2sum[:],
                                        OP.add, OP.subtract)

                # alpha = (1/S) * sum_s |yf|
                alpha = sp.tile([P, N], dt, tag="alpha")
                yv = y[:].rearrange("p (s a) -> p s a", a=N).transpose([0, 2, 1])
                nc.vector.tensor_reduce(alpha[:], yv, AX.X, OP.add,
                                        apply_absolute_value=True)
                alpha_s = sp.tile([P, N], dt, tag="alphas")
                nc.scalar.activation(alpha_s[:], alpha[:], AF.Copy,
                                     scale=1.0 / S)
                nc.sync.dma_start(out[r0:r0 + P, :], alpha_s[:])

    nc.compile()
    return nc


def _get_state():
    if "nc" not in _STATE:
        _STATE["A"] = _perm_A()
        _STATE["nc"] = _build()
    return _STATE


def _in_maps(inputs):
    st = _get_state()
    A = st["A"]
    f32 = lambda x: np.ascontiguousarray(np.asarray(x, dtype=np.float32))
    q_rows = f32(np.asarray(inputs["q_vals"], dtype=np.float32)
                 .transpose(1, 2, 0).reshape(TB, N))
    statesT = f32(np.asarray(inputs["states"], dtype=np.float32).T)
    shared = {
        "hw1_k1": f32(inputs["hw1_k1"]), "hw1_b1": f32(inputs["hw1_b1"]).reshape(1, HH),
        "hw1_k2": f32(inputs["hw1_k2"]), "hw1_b2": f32(inputs["hw1_b2"]).reshape(1, 2 * E),
        "b1_k": f32(inputs["b1_k"]), "b1_b": f32(inputs["b1_b"]).reshape(1, E),
        "hw2_k1": f32(inputs["hw2_k1"]), "hw2_b1": f32(inputs["hw2_b1"]).reshape(1, HH),
        "hw2_k2": f32(inputs["hw2_k2"]), "hw2_b2": f32(inputs["hw2_b2"]).reshape(1, E),
        "hb2_k1": f32(inputs["hb2_k1"]), "hb2_b1": f32(inputs["hb2_b1"]).reshape(1, E),
        "hb2_k2": f32(inputs["hb2_k2"]), "hb2_b2": f32(inputs["hb2_b2"]).reshape(1, 1),
    }
    maps = []
    for i in range(NCORES):
        r = slice(i * RPC, (i + 1) * RPC)
        m = dict(shared)
        m["statesT"] = f32(statesT[:, r])
        m["q"] = f32(q_rows[r])
        m["amat"] = f32(A[r])
        maps.append(m)
    return maps


def _run(inputs, trace=False, **kw):
    from concourse.bass_utils import run_bass_kernel_spmd

    st = _get_state()
    res = run_bass_kernel_spmd(st["nc"], _in_maps(inputs),
                               list(range(NCORES)), trace=trace, **kw)
    alpha = np.concatenate([res.results[i]["alpha_out"] for i in range(NCORES)],
                           axis=0)                     # (tb, n)
    full = alpha.reshape(T, B, N).transpose(2, 0, 1)   # (n, t, b)
    return np.ascontiguousarray(full.astype(np.float32)), res


def kernel(**inputs):
    out, _ = _run(inputs, trace=False)
    return out


# revision 5
# speedup vs baseline: 1.0852x; 1.0852x over previous
"""Trainium2 Bass kernel for nn_AlphaEstimate (8-core data-parallel).

Math (per row r of tb=8192 rows, sample s<16, agent a<8):
    c[r,s,a]   = sum_{a'} A[r,s,a,a'] * q[r,a']          (A: fixed perm constant)
    pre[r,s,a,e] = c[r,s,a]*w1_0[r,e] + q[r,a]*w1_1[r,e] + b1[r,e]
    h          = elu(pre);  elu(x)+1 = min(exp(x),1) + relu(x)
    y[r,s,a]   = sum_e (h+1)*w2[r,e] + b2[r] - sum_e w2[r,e]
    alpha[r,a] = mean_s |y|
where w1 = |hyper1(states)|, b1 = states@b1_k+b1_b, w2 = |hyper2(states)|,
b2 = hyper3(states). Data-parallel: 1024 rows per core, no collectives.
bf16 storage/compute (fp32 PSUM accumulation + fp32 y/tail).
"""

import sys

for _p in ("/opt/trn_rl_repo", "/opt/pypackages"):
    if _p not in sys.path:
        sys.path.append(_p)

import numpy as np

N, T, B, S = 8, 128, 64, 16
D, E, HH = 128, 64, 128
TB = T * B            # 8192
NCORES = 8
RPC = TB // NCORES    # 1024 rows per core
P = 128               # partition tile
NTILES = RPC // P     # 8 tiles per core
SA = S * N            # 128

_STATE = {}


def _perm_A():
    """Constant coalition matrix A[r, s*N*N + a*N + a'] (input independent)."""
    import jax

    cpu = jax.devices("cpu")[0]
    with jax.default_device(cpu):
        keys = jax.random.split(jax.random.key(42), TB * S)
        pos = jax.vmap(lambda k: jax.random.permutation(k, N))(keys)
        pos = np.asarray(jax.device_get(pos)).reshape(TB, S, N)
    mask = pos[:, :, None, :] < pos[:, :, :, None]          # [tb,s,a,a']
    A = mask.astype(np.float32) / np.maximum(pos, 1).astype(np.float32)[:, :, :, None]
    return np.ascontiguousarray(A.reshape(TB, S * N * N))


def _build():
    import concourse.bacc as bacc
    import concourse.tile as tile
    import concourse.mybir as mybir
    from concourse import masks

    f32 = mybir.dt.float32
    bf = mybir.dt.bfloat16
    AF = mybir.ActivationFunctionType
    OP = mybir.AluOpType
    AX = mybir.AxisListType

    nc = bacc.Bacc("TRN2", target_bir_lowering=False, debug=False,
                   num_devices=NCORES)

    def din(name, shape):
        return nc.dram_tensor(name, list(shape), bf, kind="ExternalInput").ap()

    statesT = din("statesT", (D, RPC))
    qin = din("q", (RPC, N))
    amat = din("amat", (RPC, S * N * N))
    hw1_k1 = din("hw1_k1", (D, HH))
    hw1_b1 = din("hw1_b1", (1, HH))
    hw1_k2 = din("hw1_k2", (HH, 2 * E))
    hw1_b2 = din("hw1_b2", (1, 2 * E))
    b1_k = din("b1_k", (D, E))
    b1_b = din("b1_b", (1, E))
    hw2_k1 = din("hw2_k1", (D, HH))
    hw2_b1 = din("hw2_b1", (1, HH))
    hw2_k2 = din("hw2_k2", (HH, E))
    hw2_b2 = din("hw2_b2", (1, E))
    hb2_k1 = din("hb2_k1", (D, E))
    hb2_b1 = din("hb2_b1", (1, E))
    hb2_k2 = din("hb2_k2", (E, 1))
    hb2_b2 = din("hb2_b2", (1, 1))
    out = nc.dram_tensor("alpha_out", [RPC, N], f32, kind="ExternalOutput").ap()

    with tile.TileContext(nc) as tc:
        with (
            tc.tile_pool(name="const", bufs=1) as cp,
            tc.tile_pool(name="big", bufs=2) as bp,
            tc.tile_pool(name="small", bufs=3) as sp,
            tc.tile_pool(name="psum", bufs=4, space="PSUM") as pp,
            tc.tile_pool(name="psumt", bufs=2, space="PSUM") as ppt,
        ):
            ident = cp.tile([P, P], bf)
            masks.make_identity(nc, ident[:])
            ones = cp.tile([1, P], bf)
            nc.gpsimd.memset(ones[:], 1.0)

            stT = cp.tile([D, RPC], bf)
            nc.sync.dma_start(stT[:], statesT)

            def wtile(ap, shape, _n=[0]):
                _n[0] += 1
                t = cp.tile(list(shape), bf, tag=f"wt{_n[0]}")
                nc.sync.dma_start(t[:], ap)
                return t

            W11 = wtile(hw1_k1, (D, HH)); Bi11 = wtile(hw1_b1, (1, HH))
            W12 = wtile(hw1_k2, (HH, 2 * E)); Bi12 = wtile(hw1_b2, (1, 2 * E))
            Wb1 = wtile(b1_k, (D, E)); Bib1 = wtile(b1_b, (1, E))
            W21 = wtile(hw2_k1, (D, HH)); Bi21 = wtile(hw2_b1, (1, HH))
            W22 = wtile(hw2_k2, (HH, E)); Bi22 = wtile(hw2_b2, (1, E))
            W31 = wtile(hb2_k1, (D, E)); Bi31 = wtile(hb2_b1, (1, E))
            W32 = wtile(hb2_k2, (E, 1)); Bi32 = wtile(hb2_b2, (1, 1))

            for t in range(NTILES):
                r0 = t * P
                lhs_states = stT[:, r0:r0 + P]      # (d=128, rows=128)

                def layer(wt, bi, width, act, tag, odt=bf, accum=None,
                          lhsT=None):
                    ps = pp.tile([P, width], f32, tag="ps")
                    nc.tensor.matmul(ps[:], lhsT if lhsT is not None else lhs_states,
                                     wt[:], start=True, stop=False)
                    nc.tensor.matmul(ps[:], ones[:], bi[:], start=False, stop=True)
                    sb = sp.tile([P, width], odt, tag=tag)
                    if accum is not None:
                        nc.scalar.activation(sb[:], ps[:], act, accum_out=accum)
                    else:
                        nc.scalar.activation(sb[:], ps[:], act)
                    return sb

                def transp(sb, width):
                    ps = ppt.tile([width, P], bf, tag="pst")
                    nc.tensor.transpose(ps[:], sb[:], ident[:])
                    sbT = sp.tile([width, P], bf, tag=f"tr{width}")
                    nc.scalar.activation(sbT[:], ps[:], AF.Copy)
                    return sbT

                # hypernets (feature outputs in (row, feat) orientation)
                h1 = layer(W11, Bi11, HH, AF.Relu, "hh")
                h1T = transp(h1, HH)
                w1 = layer(W12, Bi12, 2 * E, AF.Abs, "w1", lhsT=h1T[:])
                b1 = layer(Wb1, Bib1, E, AF.Copy, "b1")
                h2 = layer(W21, Bi21, HH, AF.Relu, "hh")
                h2T = transp(h2, HH)
                w2sum = sp.tile([P, 1], f32, tag="w2sum")
                w2 = layer(W22, Bi22, E, AF.Abs, "w2", accum=w2sum[:], lhsT=h2T[:])
                h3 = layer(W31, Bi31, E, AF.Relu, "h3")
                h3T = transp(h3, E)
                b2 = layer(W32, Bi32, 1, AF.Copy, "b2", odt=f32, lhsT=h3T[:])

                w1_0 = w1[:, 0:E]
                w1_1 = w1[:, E:2 * E]

                # coalition means: c = reduce_a'(A * q)
                At = bp.tile([P, S * N * N], bf, tag="amat")
                nc.sync.dma_start(At[:], amat[r0:r0 + P, :])
                qt = sp.tile([P, N], bf, tag="qt")
                nc.sync.dma_start(qt[:], qin[r0:r0 + P, :])
                prod = bp.tile([P, S * N * N], bf, tag="prod")
                nc.vector.tensor_tensor(
                    prod[:].rearrange("p (sa ap) -> p sa ap", ap=N),
                    At[:].rearrange("p (sa ap) -> p sa ap", ap=N),
                    qt[:].unsqueeze(1).broadcast_to((P, SA, N)),
                    OP.mult)
                c32 = sp.tile([P, SA], f32, tag="c32")
                nc.vector.tensor_reduce(
                    c32[:], prod[:].rearrange("p (sa ap) -> p sa ap", ap=N),
                    AX.X, OP.add)
                c = sp.tile([P, SA], bf, tag="c")
                nc.scalar.activation(c[:], c32[:], AF.Copy)

                # qw[a,e] = q_a*w1_1 + b1
                qw = sp.tile([P, N * E], bf, tag="qw")
                qwv = qw[:].rearrange("p (a e) -> p a e", e=E)
                nc.vector.tensor_tensor(
                    qwv,
                    qt[:].unsqueeze(2).broadcast_to((P, N, E)),
                    w1_1.unsqueeze(1).broadcast_to((P, N, E)),
                    OP.mult)
                nc.vector.tensor_tensor(
                    qwv, qwv,
                    b1[:].unsqueeze(1).broadcast_to((P, N, E)),
                    OP.add)

                # pre = c (x) w1_0 + qw  (sa, e) layout
                pre = bp.tile([P, SA * E], bf, tag="pre")
                pv = pre[:].rearrange("p (sa e) -> p sa e", e=E)
                nc.vector.tensor_tensor(
                    pv,
                    c[:].unsqueeze(2).broadcast_to((P, SA, E)),
                    w1_0.unsqueeze(1).broadcast_to((P, SA, E)),
                    OP.mult)
                pv4 = pre[:].rearrange("p (s a e) -> p s a e", a=N, e=E)
                nc.vector.tensor_tensor(
                    pv4, pv4,
                    qw[:].rearrange("p (a e) -> p a e", e=E)
                         .unsqueeze(1).broadcast_to((P, S, N, E)),
                    OP.add)

                # h+1 = min(exp(pre),1) + relu(pre)
                eb = bp.tile([P, SA * E], bf, tag="exp")
                nc.scalar.activation(eb[:], pre[:], AF.Exp)
                nc.scalar.activation(pre[:], pre[:], AF.Relu)
                nc.vector.scalar_tensor_tensor(
                    eb[:], eb[:], 1.0, pre[:], OP.min, OP.add)

                # y = reduce_e((h+1) * w2); yf = y + b2 - w2sum
                ev = eb[:].rearrange("p (sa e) -> p sa e", e=E)
                nc.vector.tensor_tensor(
                    ev, ev, w2[:].unsqueeze(1).broadcast_to((P, SA, E)), OP.mult)
                hv = sp.tile([P, SA * (E // 2)], bf, tag="hv")
                hvv = hv[:].rearrange("p (sa e) -> p sa e", e=E // 2)
                nc.vector.tensor_tensor(
                    hvv, eb[:].rearrange("p (sa e) -> p sa e", e=E)[:, :, 0:E // 2],
                    eb[:].rearrange("p (sa e) -> p sa e", e=E)[:, :, E // 2:E],
                    OP.add)
                y = sp.tile([P, SA], f32, tag="y")
                nc.vector.tensor_reduce(y[:], hvv, AX.X, OP.add)
                nc.vector.tensor_scalar(y[:], y[:], b2[:], w2sum[:],
                                        OP.add, OP.subtract)

                # alpha = (1/S) * sum_s |yf|
                alpha = sp.tile([P, N], f32, tag="alpha")
                yv = y[:].rearrange("p (s a) -> p s a", a=N).transpose([0, 2, 1])
                nc.vector.tensor_reduce(alpha[:], yv, AX.X, OP.add,
                                        apply_absolute_value=True)
                alpha_s = sp.tile([P, N], f32, tag="alphas")
                nc.scalar.activation(alpha_s[:], alpha[:], AF.Copy,
                                     scale=1.0 / S)
                nc.sync.dma_start(out[r0:r0 + P, :], alpha_s[:])

    nc.compile()
    return nc


def _get_state():
    if "nc" not in _STATE:
        _STATE["A"] = _perm_A()
        _STATE["nc"] = _build()
    return _STATE


def _in_maps(inputs):
    import ml_dtypes

    st = _get_state()
    A = st["A"]
    bf16 = ml_dtypes.bfloat16
    cast = lambda x: np.ascontiguousarray(
        np.asarray(x, dtype=np.float32).astype(bf16))
    q_rows = np.asarray(inputs["q_vals"], dtype=np.float32) \
        .transpose(1, 2, 0).reshape(TB, N)
    statesT = np.asarray(inputs["states"], dtype=np.float32).T
    shared = {
        "hw1_k1": cast(inputs["hw1_k1"]),
        "hw1_b1": cast(np.reshape(inputs["hw1_b1"], (1, HH))),
        "hw1_k2": cast(inputs["hw1_k2"]),
        "hw1_b2": cast(np.reshape(inputs["hw1_b2"], (1, 2 * E))),
        "b1_k": cast(inputs["b1_k"]),
        "b1_b": cast(np.reshape(inputs["b1_b"], (1, E))),
        "hw2_k1": cast(inputs["hw2_k1"]),
        "hw2_b1": cast(np.reshape(inputs["hw2_b1"], (1, HH))),
        "hw2_k2": cast(inputs["hw2_k2"]),
        "hw2_b2": cast(np.reshape(inputs["hw2_b2"], (1, E))),
        "hb2_k1": cast(inputs["hb2_k1"]),
        "hb2_b1": cast(np.reshape(inputs["hb2_b1"], (1, E))),
        "hb2_k2": cast(inputs["hb2_k2"]),
        "hb2_b2": cast(np.reshape(inputs["hb2_b2"], (1, 1))),
    }
    maps = []
    for i in range(NCORES):
        r = slice(i * RPC, (i + 1) * RPC)
        m = dict(shared)
        m["statesT"] = cast(statesT[:, r])
        m["q"] = cast(q_rows[r])
        m["amat"] = cast(A[r])
        maps.append(m)
    return maps


def _run(inputs, trace=False, **kw):
    from concourse.bass_utils import run_bass_kernel_spmd

    st = _get_state()
    res = run_bass_kernel_spmd(st["nc"], _in_maps(inputs),
                               list(range(NCORES)), trace=trace, **kw)
    alpha = np.concatenate([res.results[i]["alpha_out"] for i in range(NCORES)],
                           axis=0)                     # (tb, n)
    full = alpha.reshape(T, B, N).transpose(2, 0, 1)   # (n, t, b)
    return np.ascontiguousarray(full.astype(np.float32)), res


def kernel(**inputs):
    out, _ = _run(inputs, trace=False)
    return out


# revision 7
# speedup vs baseline: 1.1928x; 1.0992x over previous
"""Trainium2 Bass kernel for nn_AlphaEstimate (8-core data-parallel).

Math (per row r of tb=8192 rows, sample s<16, agent a<8):
    c[r,s,a]   = sum_{a'} A[r,s,a,a'] * q[r,a']          (A: fixed perm constant)
    pre[r,s,a,e] = c[r,s,a]*w1_0[r,e] + q[r,a]*w1_1[r,e] + b1[r,e]
    h          = elu(pre);  elu(x)+1 = min(exp(x),1) + relu(x)
    y[r,s,a]   = sum_e (h+1)*w2[r,e] + b2[r] - sum_e w2[r,e]
    alpha[r,a] = mean_s |y|
where w1 = |hyper1(states)|, b1 = states@b1_k+b1_b, w2 = |hyper2(states)|,
b2 = hyper3(states). Data-parallel: 1024 rows per core, no collectives.
bf16 storage/compute (fp32 PSUM accumulation + fp32 y/tail).
"""

import sys

for _p in ("/opt/trn_rl_repo", "/opt/pypackages"):
    if _p not in sys.path:
        sys.path.append(_p)

import numpy as np

N, T, B, S = 8, 128, 64, 16
D, E, HH = 128, 64, 128
TB = T * B            # 8192
NCORES = 8
RPC = TB // NCORES    # 1024 rows per core
P = 128               # partition tile
NTILES = RPC // P     # 8 tiles per core
SA = S * N            # 128

_STATE = {}


def _perm_A():
    """Constant coalition matrix A[r, s*N*N + a*N + a'] (input independent)."""
    import jax

    cpu = jax.devices("cpu")[0]
    with jax.default_device(cpu):
        keys = jax.random.split(jax.random.key(42), TB * S)
        pos = jax.vmap(lambda k: jax.random.permutation(k, N))(keys)
        pos = np.asarray(jax.device_get(pos)).reshape(TB, S, N)
    mask = pos[:, :, None, :] < pos[:, :, :, None]          # [tb,s,a,a']
    A = mask.astype(np.float32) / np.maximum(pos, 1).astype(np.float32)[:, :, :, None]
    return np.ascontiguousarray(A.reshape(TB, S * N * N))


def _build():
    import concourse.bacc as bacc
    import concourse.tile as tile
    import concourse.mybir as mybir
    from concourse import masks

    f32 = mybir.dt.float32
    bf = mybir.dt.bfloat16
    AF = mybir.ActivationFunctionType
    OP = mybir.AluOpType
    AX = mybir.AxisListType

    nc = bacc.Bacc("TRN2", target_bir_lowering=False, debug=False,
                   num_devices=NCORES)

    def din(name, shape):
        return nc.dram_tensor(name, list(shape), bf, kind="ExternalInput").ap()

    statesT = din("statesT", (D, RPC))
    qin = din("q", (RPC, N))
    amat = din("amat", (RPC, S * N * N))
    hw1_k1 = din("hw1_k1", (D, HH))
    hw1_b1 = din("hw1_b1", (1, HH))
    hw1_k2 = din("hw1_k2", (HH, 2 * E))
    hw1_b2 = din("hw1_b2", (1, 2 * E))
    b1_k = din("b1_k", (D, E))
    b1_b = din("b1_b", (1, E))
    hw2_k1 = din("hw2_k1", (D, HH))
    hw2_b1 = din("hw2_b1", (1, HH))
    hw2_k2 = din("hw2_k2", (HH, E))
    hw2_b2 = din("hw2_b2", (1, E))
    hb2_k1 = din("hb2_k1", (D, E))
    hb2_b1 = din("hb2_b1", (1, E))
    hb2_k2 = din("hb2_k2", (E, 1))
    hb2_b2 = din("hb2_b2", (1, 1))
    out = nc.dram_tensor("alpha_out", [RPC, N], f32, kind="ExternalOutput").ap()

    with tile.TileContext(nc) as tc:
        with (
            tc.tile_pool(name="const", bufs=1) as cp,
            tc.tile_pool(name="big", bufs=2) as bp,
            tc.tile_pool(name="small", bufs=3) as sp,
            tc.tile_pool(name="psum", bufs=4, space="PSUM") as pp,
            tc.tile_pool(name="psumt", bufs=2, space="PSUM") as ppt,
        ):
            ident = cp.tile([P, P], bf)
            masks.make_identity(nc, ident[:])
            ones = cp.tile([1, P], bf)
            nc.gpsimd.memset(ones[:], 1.0)

            stT = cp.tile([D, RPC], bf)
            nc.sync.dma_start(stT[:], statesT)

            def wtile(ap, shape, _n=[0]):
                _n[0] += 1
                t = cp.tile(list(shape), bf, tag=f"wt{_n[0]}")
                nc.sync.dma_start(t[:], ap)
                return t

            W11 = wtile(hw1_k1, (D, HH)); Bi11 = wtile(hw1_b1, (1, HH))
            W12 = wtile(hw1_k2, (HH, 2 * E)); Bi12 = wtile(hw1_b2, (1, 2 * E))
            Wb1 = wtile(b1_k, (D, E)); Bib1 = wtile(b1_b, (1, E))
            W21 = wtile(hw2_k1, (D, HH)); Bi21 = wtile(hw2_b1, (1, HH))
            W22 = wtile(hw2_k2, (HH, E)); Bi22 = wtile(hw2_b2, (1, E))
            W31 = wtile(hb2_k1, (D, E)); Bi31 = wtile(hb2_b1, (1, E))
            W32 = wtile(hb2_k2, (E, 1)); Bi32 = wtile(hb2_b2, (1, 1))

            for t in range(NTILES):
                r0 = t * P
                lhs_states = stT[:, r0:r0 + P]      # (d=128, rows=128)

                def layer(wt, bi, width, act, tag, odt=bf, accum=None,
                          lhsT=None):
                    ps = pp.tile([P, width], f32, tag="ps")
                    nc.tensor.matmul(ps[:], lhsT if lhsT is not None else lhs_states,
                                     wt[:], start=True, stop=False)
                    nc.tensor.matmul(ps[:], ones[:], bi[:], start=False, stop=True)
                    sb = sp.tile([P, width], odt, tag=tag)
                    if accum is not None:
                        nc.scalar.activation(sb[:], ps[:], act, accum_out=accum)
                    else:
                        nc.scalar.activation(sb[:], ps[:], act)
                    return sb

                def transp(sb, width):
                    ps = ppt.tile([width, P], bf, tag="pst")
                    nc.tensor.transpose(ps[:], sb[:], ident[:])
                    sbT = sp.tile([width, P], bf, tag=f"tr{width}")
                    nc.scalar.activation(sbT[:], ps[:], AF.Copy)
                    return sbT

                # hypernets (feature outputs in (row, feat) orientation)
                h1 = layer(W11, Bi11, HH, AF.Relu, "hh")
                h1T = transp(h1, HH)
                w1 = layer(W12, Bi12, 2 * E, AF.Abs, "w1", lhsT=h1T[:])
                b1 = layer(Wb1, Bib1, E, AF.Copy, "b1")
                h2 = layer(W21, Bi21, HH, AF.Relu, "hh")
                h2T = transp(h2, HH)
                w2sum = sp.tile([P, 1], f32, tag="w2sum")
                w2 = layer(W22, Bi22, E, AF.Abs, "w2", accum=w2sum[:], lhsT=h2T[:])
                h3 = layer(W31, Bi31, E, AF.Relu, "h3")
                h3T = transp(h3, E)
                b2 = layer(W32, Bi32, 1, AF.Copy, "b2", odt=f32, lhsT=h3T[:])

                w1_0 = w1[:, 0:E]
                w1_1 = w1[:, E:2 * E]

                # coalition means: c = reduce_a'(A * q)
                At = bp.tile([P, S * N * N], bf, tag="amat")
                nc.sync.dma_start(At[:], amat[r0:r0 + P, :])
                qt = sp.tile([P, N], bf, tag="qt")
                nc.sync.dma_start(qt[:], qin[r0:r0 + P, :])
                prod = bp.tile([P, S * N * N], bf, tag="prod")
                nc.vector.tensor_tensor(
                    prod[:].rearrange("p (sa ap) -> p sa ap", ap=N),
                    At[:].rearrange("p (sa ap) -> p sa ap", ap=N),
                    qt[:].unsqueeze(1).broadcast_to((P, SA, N)),
                    OP.mult)
                c32 = sp.tile([P, SA], f32, tag="c32")
                nc.vector.tensor_reduce(
                    c32[:], prod[:].rearrange("p (sa ap) -> p sa ap", ap=N),
                    AX.X, OP.add)
                c = sp.tile([P, SA], bf, tag="c")
                nc.scalar.activation(c[:], c32[:], AF.Copy)

                # qw[a,e] = q_a*w1_1 + b1
                qw = sp.tile([P, N * E], bf, tag="qw")
                qwv = qw[:].rearrange("p (a e) -> p a e", e=E)
                nc.vector.tensor_tensor(
                    qwv,
                    qt[:].unsqueeze(2).broadcast_to((P, N, E)),
                    w1_1.unsqueeze(1).broadcast_to((P, N, E)),
                    OP.mult)
                nc.vector.tensor_tensor(
                    qwv, qwv,
                    b1[:].unsqueeze(1).broadcast_to((P, N, E)),
                    OP.add)

                # pre = c (x) w1_0 + qw  (sa, e) layout
                # outer product on GpSimd (frees the Vector engine; the
                # broadcast APs force 1x on DVE anyway)
                pre = bp.tile([P, SA * E], bf, tag="pre")
                pv = pre[:].rearrange("p (sa e) -> p sa e", e=E)
                nc.gpsimd.tensor_tensor(
                    pv,
                    c[:].unsqueeze(2).broadcast_to((P, SA, E)),
                    w1_0.unsqueeze(1).broadcast_to((P, SA, E)),
                    OP.mult)
                pv4 = pre[:].rearrange("p (s a e) -> p s a e", a=N, e=E)
                nc.vector.tensor_tensor(
                    pv4, pv4,
                    qw[:].rearrange("p (a e) -> p a e", e=E)
                         .unsqueeze(1).broadcast_to((P, S, N, E)),
                    OP.add)

                # h+1 = min(exp(pre),1) + relu(pre)
                eb = bp.tile([P, SA * E], bf, tag="exp")
                nc.scalar.activation(eb[:], pre[:], AF.Exp)
                nc.scalar.activation(pre[:], pre[:], AF.Relu)
                nc.vector.tensor_scalar_min(eb[:], eb[:], 1.0)
                nc.vector.tensor_tensor(eb[:], eb[:], pre[:], OP.add)

                # y = reduce_e((h+1) * w2); yf = y + b2 - w2sum
                ev = eb[:].rearrange("p (sa e) -> p sa e", e=E)
                nc.vector.tensor_tensor(
                    ev, ev, w2[:].unsqueeze(1).broadcast_to((P, SA, E)), OP.mult)
                hv = sp.tile([P, SA * (E // 2)], bf, tag="hv")
                hvv = hv[:].rearrange("p (sa e) -> p sa e", e=E // 2)
                nc.vector.tensor_tensor(
                    hvv, ev[:, :, 0:E // 2], ev[:, :, E // 2:E], OP.add)
                hv2 = sp.tile([P, SA * (E // 4)], bf, tag="hv2")
                hvv2 = hv2[:].rearrange("p (sa e) -> p sa e", e=E // 4)
                nc.vector.tensor_tensor(
                    hvv2, hvv[:, :, 0:E // 4], hvv[:, :, E // 4:E // 2], OP.add)
                y = sp.tile([P, SA], f32, tag="y")
                nc.vector.tensor_reduce(y[:], hvv2, AX.X, OP.add)
                nc.vector.tensor_scalar(y[:], y[:], b2[:], w2sum[:],
                                        OP.add, OP.subtract)

                # alpha = (1/S) * sum_s |yf|
                alpha = sp.tile([P, N], f32, tag="alpha")
                yv = y[:].rearrange("p (s a) -> p s a", a=N).transpose([0, 2, 1])
                nc.vector.tensor_reduce(alpha[:], yv, AX.X, OP.add,
                                        apply_absolute_value=True)
                alpha_s = sp.tile([P, N], f32, tag="alphas")
                nc.scalar.activation(alpha_s[:], alpha[:], AF.Copy,
                                     scale=1.0 / S)
                nc.sync.dma_start(out[r0:r0 + P, :], alpha_s[:])

    nc.compile()
    return nc


def _get_state():
    if "nc" not in _STATE:
        _STATE["A"] = _perm_A()
        _STATE["nc"] = _build()
    return _STATE


def _in_maps(inputs):
    import ml_dtypes

    st = _get_state()
    A = st["A"]
    bf16 = ml_dtypes.bfloat16
    cast = lambda x: np.ascontiguousarray(
        np.asarray(x, dtype=np.float32).astype(bf16))
    q_rows = np.asarray(inputs["q_vals"], dtype=np.float32) \
        .transpose(1, 2, 0).reshape(TB, N)
    statesT = np.asarray(inputs["states"], dtype=np.float32).T
    shared = {
        "hw1_k1": cast(inputs["hw1_k1"]),
        "hw1_b1": cast(np.reshape(inputs["hw1_b1"], (1, HH))),
        "hw1_k2": cast(inputs["hw1_k2"]),
        "hw1_b2": cast(np.reshape(inputs["hw1_b2"], (1, 2 * E))),
        "b1_k": cast(inputs["b1_k"]),
        "b1_b": cast(np.reshape(inputs["b1_b"], (1, E))),
        "hw2_k1": cast(inputs["hw2_k1"]),
        "hw2_b1": cast(np.reshape(inputs["hw2_b1"], (1, HH))),
        "hw2_k2": cast(inputs["hw2_k2"]),
        "hw2_b2": cast(np.reshape(inputs["hw2_b2"], (1, E))),
        "hb2_k1": cast(inputs["hb2_k1"]),
        "hb2_b1": cast(np.reshape(inputs["hb2_b1"], (1, E))),
        "hb2_k2": cast(inputs["hb2_k2"]),
        "hb2_b2": cast(np.reshape(inputs["hb2_b2"], (1, 1))),
    }
    maps = []
    for i in range(NCORES):
        r = slice(i * RPC, (i + 1) * RPC)
        m = dict(shared)
        m["statesT"] = cast(statesT[:, r])
        m["q"] = cast(q_rows[r])
        m["amat"] = cast(A[r])
        maps.append(m)
    return maps


def _run(inputs, trace=False, **kw):
    from concourse.bass_utils import run_bass_kernel_spmd

    st = _get_state()
    res = run_bass_kernel_spmd(st["nc"], _in_maps(inputs),
                               list(range(NCORES)), trace=trace, **kw)
    alpha = np.concatenate([res.results[i]["alpha_out"] for i in range(NCORES)],
                           axis=0)                     # (tb, n)
    full = alpha.reshape(T, B, N).transpose(2, 0, 1)   # (n, t, b)
    return np.ascontiguousarray(full.astype(np.float32)), res


def kernel(**inputs):
    out, _ = _run(inputs, trace=False)
    return out


# revision 10
# speedup vs baseline: 1.6139x; 1.3531x over previous
"""Trainium2 Bass kernel for nn_AlphaEstimate (8-core data-parallel).

Math (per row r of tb=8192 rows, sample s<16, agent a<8):
    c[r,s,a]   = sum_{a'} A[r,s,a,a'] * q[r,a']          (A: fixed perm constant)
    pre[r,s,a,e] = c[r,s,a]*w1_0[r,e] + q[r,a]*w1_1[r,e] + b1[r,e]
    h          = elu(pre);  elu(x)+1 = min(exp(x),1) + relu(x)
    y[r,s,a]   = sum_e (h+1)*w2[r,e] + b2[r] - sum_e w2[r,e]
    alpha[r,a] = mean_s |y|
where w1 = |hyper1(states)|, b1 = states@b1_k+b1_b, w2 = |hyper2(states)|,
b2 = hyper3(states). Data-parallel: 1024 rows per core, no collectives.
bf16 storage/compute (fp32 PSUM accumulation + fp32 y/tail).
"""

import sys

for _p in ("/opt/trn_rl_repo", "/opt/pypackages"):
    if _p not in sys.path:
        sys.path.append(_p)

import numpy as np

N, T, B, S = 8, 128, 64, 16
D, E, HH = 128, 64, 128
TB = T * B            # 8192
NCORES = 8
RPC = TB // NCORES    # 1024 rows per core
P = 128               # partition tile
NTILES = RPC // P     # 8 tiles per core
SA = S * N            # 128

_STATE = {}


def _perm_A():
    """Constant coalition matrix A[r, s*N*N + a*N + a'] (input independent)."""
    import jax

    cpu = jax.devices("cpu")[0]
    with jax.default_device(cpu):
        keys = jax.random.split(jax.random.key(42), TB * S)
        pos = jax.vmap(lambda k: jax.random.permutation(k, N))(keys)
        pos = np.asarray(jax.device_get(pos)).reshape(TB, S, N)
    mask = pos[:, :, None, :] < pos[:, :, :, None]          # [tb,s,a,a']
    A = mask.astype(np.float32) / np.maximum(pos, 1).astype(np.float32)[:, :, :, None]
    return np.ascontiguousarray(A.reshape(TB, S * N * N))


def _build():
    import concourse.bacc as bacc
    import concourse.tile as tile
    import concourse.mybir as mybir
    from concourse import masks

    f32 = mybir.dt.float32
    bf = mybir.dt.bfloat16
    AF = mybir.ActivationFunctionType
    OP = mybir.AluOpType
    AX = mybir.AxisListType

    nc = bacc.Bacc("TRN2", target_bir_lowering=False, debug=False,
                   num_devices=NCORES)

    def din(name, shape):
        return nc.dram_tensor(name, list(shape), bf, kind="ExternalInput").ap()

    statesT = din("statesT", (D, RPC))
    qin = din("q", (RPC, N))
    amat = din("amat", (RPC, S * N * N))
    hw1_k1 = din("hw1_k1", (D, HH))
    hw1_b1 = din("hw1_b1", (1, HH))
    hw1_k2 = din("hw1_k2", (HH, 2 * E))
    hw1_b2 = din("hw1_b2", (1, 2 * E))
    b1_k = din("b1_k", (D, E))
    b1_b = din("b1_b", (1, E))
    hw2_k1 = din("hw2_k1", (D, HH))
    hw2_b1 = din("hw2_b1", (1, HH))
    hw2_k2 = din("hw2_k2", (HH, E))
    hw2_b2 = din("hw2_b2", (1, E))
    hb2_k1 = din("hb2_k1", (D, E))
    hb2_b1 = din("hb2_b1", (1, E))
    hb2_k2 = din("hb2_k2", (E, 1))
    hb2_b2 = din("hb2_b2", (1, 1))
    out = nc.dram_tensor("alpha_out", [RPC, N], f32, kind="ExternalOutput").ap()

    with tile.TileContext(nc) as tc:
        with (
            tc.tile_pool(name="const", bufs=1) as cp,
            tc.tile_pool(name="big", bufs=2) as bp,
            tc.tile_pool(name="small", bufs=3) as sp,
            tc.tile_pool(name="psum", bufs=4, space="PSUM") as pp,
            tc.tile_pool(name="psumt", bufs=2, space="PSUM") as ppt,
        ):
            ident = cp.tile([P, P], bf)
            masks.make_identity(nc, ident[:])
            ones = cp.tile([1, P], bf)
            nc.gpsimd.memset(ones[:], 1.0)

            stT = cp.tile([D, RPC], bf)
            nc.sync.dma_start(stT[:], statesT)

            def wtile(ap, shape, _n=[0]):
                _n[0] += 1
                t = cp.tile(list(shape), bf, tag=f"wt{_n[0]}")
                nc.sync.dma_start(t[:], ap)
                return t

            W11 = wtile(hw1_k1, (D, HH)); Bi11 = wtile(hw1_b1, (1, HH))
            W12 = wtile(hw1_k2, (HH, 2 * E)); Bi12 = wtile(hw1_b2, (1, 2 * E))
            Wb1 = wtile(b1_k, (D, E)); Bib1 = wtile(b1_b, (1, E))
            W21 = wtile(hw2_k1, (D, HH)); Bi21 = wtile(hw2_b1, (1, HH))
            W22 = wtile(hw2_k2, (HH, E)); Bi22 = wtile(hw2_b2, (1, E))
            W31 = wtile(hb2_k1, (D, E)); Bi31 = wtile(hb2_b1, (1, E))
            W32 = wtile(hb2_k2, (E, 1)); Bi32 = wtile(hb2_b2, (1, 1))

            for t in range(NTILES):
                r0 = t * P
                lhs_states = stT[:, r0:r0 + P]      # (d=128, rows=128)

                def layer(wt, bi, width, act, tag, odt=bf, accum=None,
                          lhsT=None):
                    ps = pp.tile([P, width], f32, tag="ps")
                    nc.tensor.matmul(ps[:], lhsT if lhsT is not None else lhs_states,
                                     wt[:], start=True, stop=False)
                    nc.tensor.matmul(ps[:], ones[:], bi[:], start=False, stop=True)
                    sb = sp.tile([P, width], odt, tag=tag)
                    if accum is not None:
                        nc.scalar.activation(sb[:], ps[:], act, accum_out=accum)
                    else:
                        nc.scalar.activation(sb[:], ps[:], act)
                    return sb

                def transp(sb, width):
                    ps = ppt.tile([width, P], bf, tag="pst")
                    nc.tensor.transpose(ps[:], sb[:], ident[:])
                    sbT = sp.tile([width, P], bf, tag=f"tr{width}")
                    nc.scalar.activation(sbT[:], ps[:], AF.Copy)
                    return sbT

                # hypernets (feature outputs in (row, feat) orientation)
                h1 = layer(W11, Bi11, HH, AF.Relu, "hh")
                h1T = transp(h1, HH)
                w1 = layer(W12, Bi12, 2 * E, AF.Abs, "w1", lhsT=h1T[:])
                b1 = layer(Wb1, Bib1, E, AF.Copy, "b1")
                h2 = layer(W21, Bi21, HH, AF.Relu, "hh")
                h2T = transp(h2, HH)
                w2sum = sp.tile([P, 1], f32, tag="w2sum")
                w2 = layer(W22, Bi22, E, AF.Abs, "w2", accum=w2sum[:], lhsT=h2T[:])
                h3 = layer(W31, Bi31, E, AF.Relu, "h3")
                h3T = transp(h3, E)
                b2 = layer(W32, Bi32, 1, AF.Copy, "b2", odt=f32, lhsT=h3T[:])

                w1_0 = w1[:, 0:E]
                w1_1 = w1[:, E:2 * E]

                # coalition means: c = reduce_a'(A * q)
                At = bp.tile([P, S * N * N], bf, tag="amat")
                nc.sync.dma_start(At[:], amat[r0:r0 + P, :])
                qt = sp.tile([P, N], bf, tag="qt")
                nc.sync.dma_start(qt[:], qin[r0:r0 + P, :])
                prod = bp.tile([P, S * N * N], bf, tag="prod")
                nc.vector.tensor_tensor(
                    prod[:].rearrange("p (sa ap) -> p sa ap", ap=N),
                    At[:].rearrange("p (sa ap) -> p sa ap", ap=N),
                    qt[:].unsqueeze(1).broadcast_to((P, SA, N)),
                    OP.mult)
                c32 = sp.tile([P, SA], f32, tag="c32")
                nc.vector.tensor_reduce(
                    c32[:], prod[:].rearrange("p (sa ap) -> p sa ap", ap=N),
                    AX.X, OP.add)
                # cast + replicate x16 so the outer product below has a
                # contiguous inner run (enables the DVE 2x perf mode)
                REP = 16
                c16 = sp.tile([P, SA * REP], bf, tag="c16")
                nc.vector.tensor_copy(
                    c16[:].rearrange("p (sa r) -> p sa r", r=REP),
                    c32[:].unsqueeze(2).broadcast_to((P, SA, REP)))

                # qw[a,e] = q_a*w1_1 + b1
                qw = sp.tile([P, N * E], bf, tag="qw")
                qwv = qw[:].rearrange("p (a e) -> p a e", e=E)
                nc.vector.tensor_tensor(
                    qwv,
                    qt[:].unsqueeze(2).broadcast_to((P, N, E)),
                    w1_1.unsqueeze(1).broadcast_to((P, N, E)),
                    OP.mult)
                nc.vector.tensor_tensor(
                    qwv, qwv,
                    b1[:].unsqueeze(1).broadcast_to((P, N, E)),
                    OP.add)

                # pre = c (x) w1_0 + qw  (sa, e) layout; both operands have
                # contiguous 16-wide inner runs -> DVE 2x mode
                pre = bp.tile([P, SA * E], bf, tag="pre")
                pv4d = pre[:].rearrange("p (sa eh el) -> p sa eh el",
                                        eh=E // REP, el=REP)
                nc.vector.tensor_tensor(
                    pv4d,
                    c16[:].rearrange("p (sa r) -> p sa r", r=REP)
                          .unsqueeze(2).broadcast_to((P, SA, E // REP, REP)),
                    w1_0.rearrange("p (eh el) -> p eh el", el=REP)
                        .unsqueeze(1).broadcast_to((P, SA, E // REP, REP)),
                    OP.mult)
                pv4 = pre[:].rearrange("p (s a e) -> p s a e", a=N, e=E)
                nc.vector.tensor_tensor(
                    pv4, pv4,
                    qw[:].rearrange("p (a e) -> p a e", e=E)
                         .unsqueeze(1).broadcast_to((P, S, N, E)),
                    OP.add)

                # h+1 = min(exp(pre),1) + relu(pre)
                eb = bp.tile([P, SA * E], bf, tag="exp")
                nc.scalar.activation(eb[:], pre[:], AF.Exp)
                nc.scalar.activation(pre[:], pre[:], AF.Relu)
                nc.vector.tensor_scalar_min(eb[:], eb[:], 1.0)
                nc.vector.tensor_tensor(eb[:], eb[:], pre[:], OP.add)

                # y = reduce_e((h+1) * w2); yf = y + b2 - w2sum
                ev = eb[:].rearrange("p (sa e) -> p sa e", e=E)
                nc.vector.tensor_tensor(
                    ev, ev, w2[:].unsqueeze(1).broadcast_to((P, SA, E)), OP.mult)
                hv = sp.tile([P, SA * (E // 2)], bf, tag="hv")
                hvv = hv[:].rearrange("p (sa e) -> p sa e", e=E // 2)
                nc.vector.tensor_tensor(
                    hvv, ev[:, :, 0:E // 2], ev[:, :, E // 2:E], OP.add)
                hv2 = sp.tile([P, SA * (E // 4)], bf, tag="hv2")
                hvv2 = hv2[:].rearrange("p (sa e) -> p sa e", e=E // 4)
                nc.vector.tensor_tensor(
                    hvv2, hvv[:, :, 0:E // 4], hvv[:, :, E // 4:E // 2], OP.add)
                y = sp.tile([P, SA], f32, tag="y")
                nc.vector.tensor_reduce(y[:], hvv2, AX.X, OP.add)
                nc.vector.tensor_scalar(y[:], y[:], b2[:], w2sum[:],
                                        OP.add, OP.subtract)

                # alpha = (1/S) * sum_s |yf|
                alpha = sp.tile([P, N], f32, tag="alpha")
                yv = y[:].rearrange("p (s a) -> p s a", a=N).transpose([0, 2, 1])
                nc.vector.tensor_reduce(alpha[:], yv, AX.X, OP.add,
                                        apply_absolute_value=True)
                alpha_s = sp.tile([P, N], f32, tag="alphas")
                nc.scalar.activation(alpha_s[:], alpha[:], AF.Copy,
                                     scale=1.0 / S)
                nc.sync.dma_start(out[r0:r0 + P, :], alpha_s[:])

    nc.compile()
    return nc


def _get_state():
    if "nc" not in _STATE:
        _STATE["A"] = _perm_A()
        _STATE["nc"] = _build()
    return _STATE


def _in_maps(inputs):
    import ml_dtypes

    st = _get_state()
    A = st["A"]
    bf16 = ml_dtypes.bfloat16
    cast = lambda x: np.ascontiguousarray(
        np.asarray(x, dtype=np.float32).astype(bf16))
    q_rows = np.asarray(inputs["q_vals"], dtype=np.float32) \
        .transpose(1, 2, 0).reshape(TB, N)
    statesT = np.asarray(inputs["states"], dtype=np.float32).T
    shared = {
        "hw1_k1": cast(inputs["hw1_k1"]),
        "hw1_b1": cast(np.reshape(inputs["hw1_b1"], (1, HH))),
        "hw1_k2": cast(inputs["hw1_k2"]),
        "hw1_b2": cast(np.reshape(inputs["hw1_b2"], (1, 2 * E))),
        "b1_k": cast(inputs["b1_k"]),
        "b1_b": cast(np.reshape(inputs["b1_b"], (1, E))),
        "hw2_k1": cast(inputs["hw2_k1"]),
        "hw2_b1": cast(np.reshape(inputs["hw2_b1"], (1, HH))),
        "hw2_k2": cast(inputs["hw2_k2"]),
        "hw2_b2": cast(np.reshape(inputs["hw2_b2"], (1, E))),
        "hb2_k1": cast(inputs["hb2_k1"]),
        "hb2_b1": cast(np.reshape(inputs["hb2_b1"], (1, E))),
        "hb2_k2": cast(inputs["hb2_k2"]),
        "hb2_b2": cast(np.reshape(inputs["hb2_b2"], (1, 1))),
    }
    maps = []
    for i in range(NCORES):
        r = slice(i * RPC, (i + 1) * RPC)
        m = dict(shared)
        m["statesT"] = cast(statesT[:, r])
        m["q"] = cast(q_rows[r])
        m["amat"] = cast(A[r])
        maps.append(m)
    return maps


def _run(inputs, trace=False, **kw):
    from concourse.bass_utils import run_bass_kernel_spmd

    st = _get_state()
    res = run_bass_kernel_spmd(st["nc"], _in_maps(inputs),
                               list(range(NCORES)), trace=trace, **kw)
    alpha = np.concatenate([res.results[i]["alpha_out"] for i in range(NCORES)],
                           axis=0)                     # (tb, n)
    full = alpha.reshape(T, B, N).transpose(2, 0, 1)   # (n, t, b)
    return np.ascontiguousarray(full.astype(np.float32)), res


def kernel(**inputs):
    out, _ = _run(inputs, trace=False)
    return out


# revision 13
# speedup vs baseline: 1.6399x; 1.0161x over previous
"""Trainium2 Bass kernel for nn_AlphaEstimate (8-core data-parallel).

Math (per row r of tb=8192 rows, sample s<16, agent a<8):
    c[r,s,a]   = sum_{a'} A[r,s,a,a'] * q[r,a']          (A: fixed perm constant)
    pre[r,s,a,e] = c[r,s,a]*w1_0[r,e] + q[r,a]*w1_1[r,e] + b1[r,e]
    h          = elu(pre);  elu(x)+1 = min(exp(x),1) + relu(x)
    y[r,s,a]   = sum_e (h+1)*w2[r,e] + b2[r] - sum_e w2[r,e]
    alpha[r,a] = mean_s |y|
where w1 = |hyper1(states)|, b1 = states@b1_k+b1_b, w2 = |hyper2(states)|,
b2 = hyper3(states). Data-parallel: 1024 rows per core, no collectives.
bf16 storage/compute (fp32 PSUM accumulation + fp32 y/tail).
"""

import sys

for _p in ("/opt/trn_rl_repo", "/opt/pypackages"):
    if _p not in sys.path:
        sys.path.append(_p)

import numpy as np

N, T, B, S = 8, 128, 64, 16
D, E, HH = 128, 64, 128
TB = T * B            # 8192
NCORES = 8
RPC = TB // NCORES    # 1024 rows per core
P = 128               # partition tile
NTILES = RPC // P     # 8 tiles per core
SA = S * N            # 128

_STATE = {}


def _perm_A():
    """Constant coalition matrix A[r, s*N*N + a*N + a'] (input independent)."""
    import jax

    cpu = jax.devices("cpu")[0]
    with jax.default_device(cpu):
        keys = jax.random.split(jax.random.key(42), TB * S)
        pos = jax.vmap(lambda k: jax.random.permutation(k, N))(keys)
        pos = np.asarray(jax.device_get(pos)).reshape(TB, S, N)
    mask = pos[:, :, None, :] < pos[:, :, :, None]          # [tb,s,a,a']
    A = mask.astype(np.float32) / np.maximum(pos, 1).astype(np.float32)[:, :, :, None]
    return np.ascontiguousarray(A.reshape(TB, S * N * N))


def _build():
    import concourse.bacc as bacc
    import concourse.tile as tile
    import concourse.mybir as mybir
    from concourse import masks

    f32 = mybir.dt.float32
    bf = mybir.dt.bfloat16
    AF = mybir.ActivationFunctionType
    OP = mybir.AluOpType
    AX = mybir.AxisListType

    nc = bacc.Bacc("TRN2", target_bir_lowering=False, debug=False,
                   num_devices=NCORES)

    def din(name, shape):
        return nc.dram_tensor(name, list(shape), bf, kind="ExternalInput").ap()

    statesT = din("statesT", (D, RPC))
    qin = din("q", (RPC, N))
    amat = din("amat", (RPC, S * N * N))
    hw1_k1 = din("hw1_k1", (D, HH))
    hw1_b1 = din("hw1_b1", (1, HH))
    hw1_k2 = din("hw1_k2", (HH, 2 * E))
    hw1_b2 = din("hw1_b2", (1, 2 * E))
    b1_k = din("b1_k", (D, E))
    b1_b = din("b1_b", (1, E))
    hw2_k1 = din("hw2_k1", (D, HH))
    hw2_b1 = din("hw2_b1", (1, HH))
    hw2_k2 = din("hw2_k2", (HH, E))
    hw2_b2 = din("hw2_b2", (1, E))
    hb2_k1 = din("hb2_k1", (D, E))
    hb2_b1 = din("hb2_b1", (1, E))
    hb2_k2 = din("hb2_k2", (E, 1))
    hb2_b2 = din("hb2_b2", (1, 1))
    out = nc.dram_tensor("alpha_out", [RPC, N], f32, kind="ExternalOutput").ap()

    with tile.TileContext(nc) as tc:
        with (
            tc.tile_pool(name="const", bufs=1) as cp,
            tc.tile_pool(name="big", bufs=2) as bp,
            tc.tile_pool(name="small", bufs=3) as sp,
            tc.tile_pool(name="psum", bufs=4, space="PSUM") as pp,
            tc.tile_pool(name="psumt", bufs=2, space="PSUM") as ppt,
        ):
            ident = cp.tile([P, P], bf)
            masks.make_identity(nc, ident[:])
            ones = cp.tile([1, P], bf)
            nc.gpsimd.memset(ones[:], 1.0)

            stT = cp.tile([D, RPC], bf)
            nc.sync.dma_start(stT[:], statesT)

            def wtile(ap, shape, _n=[0]):
                _n[0] += 1
                t = cp.tile(list(shape), bf, tag=f"wt{_n[0]}")
                nc.sync.dma_start(t[:], ap)
                return t

            W11 = wtile(hw1_k1, (D, HH)); Bi11 = wtile(hw1_b1, (1, HH))
            W12 = wtile(hw1_k2, (HH, 2 * E)); Bi12 = wtile(hw1_b2, (1, 2 * E))
            Wb1 = wtile(b1_k, (D, E)); Bib1 = wtile(b1_b, (1, E))
            W21 = wtile(hw2_k1, (D, HH)); Bi21 = wtile(hw2_b1, (1, HH))
            W22 = wtile(hw2_k2, (HH, E)); Bi22 = wtile(hw2_b2, (1, E))
            W31 = wtile(hb2_k1, (D, E)); Bi31 = wtile(hb2_b1, (1, E))
            W32 = wtile(hb2_k2, (E, 1)); Bi32 = wtile(hb2_b2, (1, 1))

            for t in range(NTILES):
                r0 = t * P
                lhs_states = stT[:, r0:r0 + P]      # (d=128, rows=128)

                def layer(wt, bi, width, act, tag, odt=bf, accum=None,
                          lhsT=None):
                    ps = pp.tile([P, width], f32, tag="ps")
                    nc.tensor.matmul(ps[:], lhsT if lhsT is not None else lhs_states,
                                     wt[:], start=True, stop=False)
                    nc.tensor.matmul(ps[:], ones[:], bi[:], start=False, stop=True)
                    sb = sp.tile([P, width], odt, tag=tag)
                    if accum is not None:
                        nc.scalar.activation(sb[:], ps[:], act, accum_out=accum)
                    else:
                        nc.scalar.activation(sb[:], ps[:], act)
                    return sb

                def transp(sb, width):
                    ps = ppt.tile([width, P], bf, tag="pst")
                    nc.tensor.transpose(ps[:], sb[:], ident[:])
                    sbT = sp.tile([width, P], bf, tag=f"tr{width}")
                    nc.scalar.activation(sbT[:], ps[:], AF.Copy)
                    return sbT

                # hypernets (feature outputs in (row, feat) orientation)
                h1 = layer(W11, Bi11, HH, AF.Relu, "hh")
                h1T = transp(h1, HH)
                w1 = layer(W12, Bi12, 2 * E, AF.Abs, "w1", lhsT=h1T[:])
                b1 = layer(Wb1, Bib1, E, AF.Copy, "b1")
                h2 = layer(W21, Bi21, HH, AF.Relu, "hh")
                h2T = transp(h2, HH)
                w2sum = sp.tile([P, 1], f32, tag="w2sum")
                w2 = layer(W22, Bi22, E, AF.Abs, "w2", accum=w2sum[:], lhsT=h2T[:])
                h3 = layer(W31, Bi31, E, AF.Relu, "h3")
                h3T = transp(h3, E)
                b2 = layer(W32, Bi32, 1, AF.Copy, "b2", odt=f32, lhsT=h3T[:])

                w1_0 = w1[:, 0:E]
                w1_1 = w1[:, E:2 * E]

                # coalition means: c = reduce_a'(A * q)
                At = bp.tile([P, S * N * N], bf, tag="amat")
                nc.sync.dma_start(At[:], amat[r0:r0 + P, :])
                qt = sp.tile([P, N], bf, tag="qt")
                nc.sync.dma_start(qt[:], qin[r0:r0 + P, :])
                prod = bp.tile([P, S * N * N], bf, tag="prod")
                nc.vector.tensor_tensor(
                    prod[:].rearrange("p (sa ap) -> p sa ap", ap=N),
                    At[:].rearrange("p (sa ap) -> p sa ap", ap=N),
                    qt[:].unsqueeze(1).broadcast_to((P, SA, N)),
                    OP.mult)
                c32 = sp.tile([P, SA], f32, tag="c32")
                nc.vector.tensor_reduce(
                    c32[:], prod[:].rearrange("p (sa ap) -> p sa ap", ap=N),
                    AX.X, OP.add)
                # cast + replicate x16 so the outer product below has a
                # contiguous inner run (enables the DVE 2x perf mode)
                REP = 16
                c16 = sp.tile([P, SA * REP], bf, tag="c16")
                nc.vector.tensor_copy(
                    c16[:].rearrange("p (sa r) -> p sa r", r=REP),
                    c32[:].unsqueeze(2).broadcast_to((P, SA, REP)))

                # qw[a,e] = q_a*w1_1 + b1
                qw = sp.tile([P, N * E], bf, tag="qw")
                qwv = qw[:].rearrange("p (a e) -> p a e", e=E)
                nc.vector.tensor_tensor(
                    qwv,
                    qt[:].unsqueeze(2).broadcast_to((P, N, E)),
                    w1_1.unsqueeze(1).broadcast_to((P, N, E)),
                    OP.mult)
                nc.vector.tensor_tensor(
                    qwv, qwv,
                    b1[:].unsqueeze(1).broadcast_to((P, N, E)),
                    OP.add)

                # pre = c (x) w1_0 + qw  (sa, e) layout; both operands have
                # contiguous 16-wide inner runs -> DVE 2x mode
                pre = bp.tile([P, SA * E], bf, tag="pre")
                pv4d = pre[:].rearrange("p (sa eh el) -> p sa eh el",
                                        eh=E // REP, el=REP)
                nc.vector.tensor_tensor(
                    pv4d,
                    c16[:].rearrange("p (sa r) -> p sa r", r=REP)
                          .unsqueeze(2).broadcast_to((P, SA, E // REP, REP)),
                    w1_0.rearrange("p (eh el) -> p eh el", el=REP)
                        .unsqueeze(1).broadcast_to((P, SA, E // REP, REP)),
                    OP.mult)
                pv4 = pre[:].rearrange("p (s a e) -> p s a e", a=N, e=E)
                nc.vector.tensor_tensor(
                    pv4, pv4,
                    qw[:].rearrange("p (a e) -> p a e", e=E)
                         .unsqueeze(1).broadcast_to((P, S, N, E)),
                    OP.add)

                # h+1 = min(exp(pre),1) + relu(pre)
                eb = bp.tile([P, SA * E], bf, tag="exp")
                nc.scalar.activation(eb[:], pre[:], AF.Exp)
                nc.scalar.activation(pre[:], pre[:], AF.Relu)
                nc.vector.tensor_scalar_min(eb[:], eb[:], 1.0)
                nc.vector.tensor_tensor(eb[:], eb[:], pre[:], OP.add)

                # y = reduce_e((h+1) * w2); yf = y + b2 - w2sum
                ev = eb[:].rearrange("p (sa e) -> p sa e", e=E)
                nc.vector.tensor_tensor(
                    ev, ev, w2[:].unsqueeze(1).broadcast_to((P, SA, E)), OP.mult)
                hv = sp.tile([P, SA * (E // 2)], bf, tag="hv")
                hvv = hv[:].rearrange("p (sa e) -> p sa e", e=E // 2)
                nc.vector.tensor_tensor(
                    hvv, ev[:, :, 0:E // 2], ev[:, :, E // 2:E], OP.add)
                hv2 = sp.tile([P, SA * (E // 4)], bf, tag="hv2")
                hvv2 = hv2[:].rearrange("p (sa e) -> p sa e", e=E // 4)
                nc.vector.tensor_tensor(
                    hvv2, hvv[:, :, 0:E // 4], hvv[:, :, E // 4:E // 2], OP.add)
                hv3 = sp.tile([P, SA * (E // 8)], bf, tag="hv3")
                hvv3 = hv3[:].rearrange("p (sa e) -> p sa e", e=E // 8)
                nc.vector.tensor_tensor(
                    hvv3, hvv2[:, :, 0:E // 8], hvv2[:, :, E // 8:E // 4], OP.add)
                y = sp.tile([P, SA], f32, tag="y")
                nc.vector.tensor_reduce(y[:], hvv3, AX.X, OP.add)
                nc.vector.tensor_scalar(y[:], y[:], b2[:], w2sum[:],
                                        OP.add, OP.subtract)

                # alpha = (1/S) * sum_s |yf|
                alpha = sp.tile([P, N], f32, tag="alpha")
                yv = y[:].rearrange("p (s a) -> p s a", a=N).transpose([0, 2, 1])
                nc.vector.tensor_reduce(alpha[:], yv, AX.X, OP.add,
                                        apply_absolute_value=True)
                alpha_s = sp.tile([P, N], f32, tag="alphas")
                nc.scalar.activation(alpha_s[:], alpha[:], AF.Copy,
                                     scale=1.0 / S)
                nc.sync.dma_start(out[r0:r0 + P, :], alpha_s[:])

    nc.compile()
    return nc


def _get_state():
    if "nc" not in _STATE:
        _STATE["A"] = _perm_A()
        _STATE["nc"] = _build()
    return _STATE


def _in_maps(inputs):
    import ml_dtypes

    st = _get_state()
    A = st["A"]
    bf16 = ml_dtypes.bfloat16
    cast = lambda x: np.ascontiguousarray(
        np.asarray(x, dtype=np.float32).astype(bf16))
    q_rows = np.asarray(inputs["q_vals"], dtype=np.float32) \
        .transpose(1, 2, 0).reshape(TB, N)
    statesT = np.asarray(inputs["states"], dtype=np.float32).T
    shared = {
        "hw1_k1": cast(inputs["hw1_k1"]),
        "hw1_b1": cast(np.reshape(inputs["hw1_b1"], (1, HH))),
        "hw1_k2": cast(inputs["hw1_k2"]),
        "hw1_b2": cast(np.reshape(inputs["hw1_b2"], (1, 2 * E))),
        "b1_k": cast(inputs["b1_k"]),
        "b1_b": cast(np.reshape(inputs["b1_b"], (1, E))),
        "hw2_k1": cast(inputs["hw2_k1"]),
        "hw2_b1": cast(np.reshape(inputs["hw2_b1"], (1, HH))),
        "hw2_k2": cast(inputs["hw2_k2"]),
        "hw2_b2": cast(np.reshape(inputs["hw2_b2"], (1, E))),
        "hb2_k1": cast(inputs["hb2_k1"]),
        "hb2_b1": cast(np.reshape(inputs["hb2_b1"], (1, E))),
        "hb2_k2": cast(inputs["hb2_k2"]),
        "hb2_b2": cast(np.reshape(inputs["hb2_b2"], (1, 1))),
    }
    maps = []
    for i in range(NCORES):
        r = slice(i * RPC, (i + 1) * RPC)
        m = dict(shared)
        m["statesT"] = cast(statesT[:, r])
        m["q"] = cast(q_rows[r])
        m["amat"] = cast(A[r])
        maps.append(m)
    return maps


def _run(inputs, trace=False, **kw):
    from concourse.bass_utils import run_bass_kernel_spmd

    st = _get_state()
    res = run_bass_kernel_spmd(st["nc"], _in_maps(inputs),
                               list(range(NCORES)), trace=trace, **kw)
    alpha = np.concatenate([res.results[i]["alpha_out"] for i in range(NCORES)],
                           axis=0)                     # (tb, n)
    full = alpha.reshape(T, B, N).transpose(2, 0, 1)   # (n, t, b)
    return np.ascontiguousarray(full.astype(np.float32)), res


def kernel(**inputs):
    out, _ = _run(inputs, trace=False)
    return out
